# revision 9
# baseline (speedup 1.0000x reference)
"""Trainium2 Bass kernel for a dense transformer block (B=2, T=2048, C=1024,
H=16, DFF=4096), distributed over 8 NeuronCores.

Sharding: 2 batch groups x 4-way query-block sharding. Core c handles batch
g=c//4 and query blocks {j, 7-j} (j=c%4) of 8 blocks of 256 rows. K/V are
computed per-core for the full batch (replicated; no collectives). Causality
is exploited statically: key-chunks 0-7 are needed by both query blocks
(masked only on block-0's columns), chunks 8-15 only by the late block.
The data-dependent causal boundary is applied with per-core 0/1 masks so one
NEFF serves all 8 cores (SPMD).

Activations are kept feature-major ("xT") so every matmul chains without
transposes; layernorm runs row-major with PE transposes between domains.
Softmax denominators ride the AV matmul as an extra ones-column of V.
"""
import numpy as np
import ml_dtypes

import concourse.bass as bass
import concourse.mybir as mybir
import concourse.tile as tile
from concourse.vector_clock import ScopedClock
from concourse.bass_utils import run_bass_kernel_spmd
from concourse.masks import make_identity

bf16 = ml_dtypes.bfloat16
f32 = mybir.dt.float32
bt16 = mybir.dt.bfloat16
AF = mybir.ActivationFunctionType
OP = mybir.AluOpType

B, T, C, H, DH, DFF = 2, 2048, 1024, 16, 64, 4096
P = 128
QB = 256            # rows per query block
R = 512             # own query rows per core
RT = T + R          # ln1 rows per core (full batch + own q rows)
CC = C // P         # 8 feature chunks
MM = DFF // P       # 32 ffn chunks
EPS = 1e-5


# ---------------------------------------------------------------------------
# The walrus build in this container rejects instructions with >1 sync wait.
# Tile's sem assignment can emit several on one instruction; split the excess
# onto same-engine NoOps placed immediately before.
def _patched_drain_and_barrier(self, tick_clock, wait_clock):
    nc = self.nc
    probe = nc.sync.nop(nofuse=True, hint="tail_wait_probe")
    wait_clock.add_sem_waits(probe.ins, ScopedClock({None: tick_clock.global_clock}))
    si = probe.ins.sync_info
    waits = list(si.on_wait) if si is not None else []
    if si is not None:
        si.on_wait = waits[:1]
    for w in waits[1:]:
        n2 = nc.sync.nop(nofuse=True, hint="tail_wait_split")
        n2.ins.sync_info = mybir.SyncInfo(on_wait=[w], on_update=[])
    nc.sync.drain()
    nc.all_engine_barrier()
    assert self.sems is not None
    popped = nc._tile_sem_poison_stack.pop()
    assert popped is self._sem_poison
    nc.clear_and_free_semaphores(list(self.sems.allocated().values()))
    nc.all_engine_barrier()


tile.TileContext._drain_and_barrier = _patched_drain_and_barrier

_MAX_WAITS = 1
_split_counter = [0]


def _split_sync_waits(nc):
    for fn in nc.m.functions:
        for bb in fn.blocks:
            new_insts = []
            for inst in bb.instructions:
                si = getattr(inst, "sync_info", None)
                lim = _MAX_WAITS
                if si is not None and si.on_wait and len(si.on_wait) > lim:
                    waits = list(si.on_wait)
                    keep = waits[-lim:]
                    excess = waits[:-lim]
                    for i in range(0, len(excess), _MAX_WAITS):
                        _split_counter[0] += 1
                        nop = mybir.InstNoOp(
                            name=f"I-wsplit-{_split_counter[0]}", ins=[], outs=[])
                        nop.engine = inst.engine
                        nop.sync_info = mybir.SyncInfo(
                            on_wait=excess[i:i + _MAX_WAITS], on_update=[])
                        new_insts.append(nop)
                    si.on_wait = keep
                new_insts.append(inst)
            bb.instructions = new_insts
# ---------------------------------------------------------------------------


class Ctx:
    pass


def _layernorm_tile(g, xt, out_writes):
    """Row-major LN of xt [128, C] (in place), then transpose chunks and run
    out_writes(c, psum_ap) for each feature chunk c."""
    nc = g.nc
    st = g.stats.tile([P, 2, 6], f32, tag="bnst", name="bnst")
    xv = xt.rearrange("p (s d) -> p s d", s=2)
    for sg in range(2):
        nc.vector.bn_stats(out=st[:, sg, :], in_=xv[:, sg, :])
    mv = g.stats.tile([P, 2], f32, tag="bnmv", name="bnmv")
    nc.vector.bn_aggr(out=mv[:], in_=st[:])
    sq = g.stats.tile([P, 1], f32, tag="bnsq", name="bnsq")
    nc.scalar.activation(out=sq[:], in_=mv[:, 1:2], func=AF.Sqrt,
                         bias=g.eps_sb[:], scale=float(C) / (C - 1))
    rstd = g.stats.tile([P, 1], f32, tag="bnrstd", name="bnrstd")
    nc.vector.reciprocal(rstd[:], sq[:])
    nc.vector.tensor_scalar(out=xt[:], in0=xt[:], scalar1=mv[:, 0:1],
                            scalar2=rstd[:], op0=OP.subtract, op1=OP.mult)
    for c in range(CC):
        pt = g.ps.tile([P, 512], f32, tag="ps", name="ps_t")
        nc.tensor.transpose(pt[:P, :P], xt[:, c * P:(c + 1) * P], g.ident[:])
        out_writes(c, pt[:P, :P])


def _phase_a(g):
    """LN1 + transpose + Q/K/V projections."""
    nc, tc = g.nc, g.tc
    with tc.tile_pool(name="x1p", bufs=1) as x1p, \
         tc.tile_pool(name="xio", bufs=3) as xio, \
         tc.tile_pool(name="wvp", bufs=1) as wvp:
        # x1T split per 512-row block (rb 0-3 = batch, rb 4 = own q rows)
        x1T = [x1p.tile([P, CC, 512], bt16, tag=f"x1T{rb}", name=f"x1T{rb}")
               for rb in range(5)]
        for kt in range(T // P):
            nc.vector.memset(g.vv[kt][:, :, DH:DH + 1], 1.0)

        for rt in range(RT // P):
            rb, r0 = rt // 4, (rt % 4) * P
            xt = xio.tile([P, C], f32, tag="xin", name="xin")
            nc.sync.dma_start(xt[:], g.xc[rt * P:(rt + 1) * P, :])

            def wr1(c, pt, rb=rb, r0=r0, rt=rt):
                nc.scalar.activation(
                    out=x1T[rb][:, c, r0:r0 + P], in_=pt, func=AF.Identity,
                    bias=g.be1s[:, c:c + 1], scale=g.g1s[:, c:c + 1])
                if rt >= T // P:
                    q0 = (rt - T // P) * P
                    nc.scalar.activation(
                        out=g.x1f[c][:, q0:q0 + P], in_=pt, func=AF.Identity,
                        bias=g.be1s[:, c:c + 1], scale=g.g1s[:, c:c + 1])
            _layernorm_tile(g, xt, wr1)

        # K^T and Q^T projections (feature-major out, weights as lhsT)
        for m in range(CC):
            wkm = g.wstr.tile([P, CC, P], bt16, tag="wstr", name="wkm")
            nc.sync.dma_start(
                wkm[:, :, :],
                g.wk[:, m * P:(m + 1) * P].rearrange("(c p) f -> p c f", p=P))
            for rb in range(4):
                pk = g.ps.tile([P, 512], f32, tag="ps", name="ps_k")
                for c in range(CC):
                    nc.tensor.matmul(pk[:], wkm[:, c, :], x1T[rb][:, c, :],
                                     start=(c == 0), stop=(c == CC - 1))
                nc.vector.tensor_scalar(
                    out=g.kT[m][rb][:, :], in0=pk[:],
                    scalar1=g.sb_vec["bk"][:, m:m + 1], scalar2=None, op0=OP.add)
            wqm = g.wstr.tile([P, CC, P], bt16, tag="wstr", name="wqm")
            nc.sync.dma_start(
                wqm[:, :, :],
                g.wq[:, m * P:(m + 1) * P].rearrange("(c p) f -> p c f", p=P))
            pq = g.ps.tile([P, 512], f32, tag="ps", name="ps_q")
            for c in range(CC):
                nc.tensor.matmul(pq[:], wqm[:, c, :], x1T[4][:, c, :],
                                 start=(c == 0), stop=(c == CC - 1))
            nc.vector.tensor_scalar(
                out=g.qT[m][:, :], in0=pq[:],
                scalar1=g.sb_vec["bq"][:, m:m + 1], scalar2=None, op0=OP.add)

        # V row-major (keys on partitions): lhsT = x1T chunk, rhs = wv
        wvs = wvp.tile([P, CC, C], bt16, tag="wvs", name="wvs")
        nc.sync.dma_start(wvs[:], g.wv.rearrange("(c p) f -> p c f", p=P))
        for kt in range(T // P):
            rb, r0 = kt // 4, (kt % 4) * P
            for half in range(2):
                pv = g.ps.tile([P, 512], f32, tag="ps", name="ps_v")
                for c in range(CC):
                    nc.tensor.matmul(pv[:], x1T[rb][:, c, r0:r0 + P],
                                     wvs[:, c, half * 512:(half + 1) * 512],
                                     start=(c == 0), stop=(c == CC - 1))
                nc.vector.tensor_copy(
                    out=g.vv[kt][:, half * 8:(half + 1) * 8, 0:DH],
                    in_=pv.rearrange("p (h d) -> p h d", h=8))


def _phase_b(g):
    """Attention, both query blocks fused on the free dim (cols 0:256 = early
    block, 256:512 = late block). Key-chunks 0-7 feed both blocks (one N=512
    matmul); chunks 8-15 feed only the late block (N=256)."""
    nc, tc = g.nc, g.tc
    with tc.tile_pool(name="mp", bufs=1) as mp, \
         tc.tile_pool(name="apl", bufs=2) as apl:
        mq = mp.tile([P, 16, QB], bt16, tag="mask", name="mask")
        nc.sync.dma_start(mq[:], g.masks)
        for pair in range(CC):
            for hl in range(2):
                h = 2 * pair + hl
                hs = slice(hl * DH, (hl + 1) * DH)
                aA = apl.tile([P, 8, 512], bt16, tag="aA", name="aA")
                aB = apl.tile([P, 8, QB], bt16, tag="aB", name="aB")
                for kc in range(16):
                    rb, k0 = kc // 4, (kc % 4) * P
                    psc = g.ps.tile([P, 512], f32, tag="ps", name="ps_s")
                    n = 512 if kc < 8 else QB
                    q0 = 0 if kc < 8 else QB
                    nc.tensor.matmul(
                        psc[:, :n], g.kT[pair][rb][hs, k0:k0 + P],
                        g.qT[pair][hs, q0:512],
                        start=True, stop=True, tile_position=(hl * DH, 0))
                    if kc < 8:
                        nc.scalar.activation(out=aA[:, kc, :], in_=psc[:, :512],
                                             func=AF.Exp)
                        nc.vector.tensor_mul(aA[:, kc, 0:QB], aA[:, kc, 0:QB],
                                             mq[:, kc, :])
                    else:
                        nc.scalar.activation(out=aB[:, kc - 8, :],
                                             in_=psc[:, :QB], func=AF.Exp)
                        nc.vector.tensor_mul(aB[:, kc - 8, :], aB[:, kc - 8, :],
                                             mq[:, kc, :])
                pav = g.ps.tile([P, 512], f32, tag="ps", name="ps_av")
                for kc in range(8):
                    nc.tensor.matmul(pav[:DH + 1, :512], g.vv[kc][:, h, :],
                                     aA[:, kc, :], start=(kc == 0), stop=False)
                for kc in range(8, 16):
                    nc.tensor.matmul(pav[:DH + 1, QB:512], g.vv[kc][:, h, :],
                                     aB[:, kc - 8, :], start=False,
                                     stop=(kc == 15))
                den = g.stats.tile([1, 512], f32, tag="den", name="den")
                nc.vector.tensor_copy(den[:], pav[DH:DH + 1, :512])
                rr = g.stats.tile([1, 512], f32, tag="rr", name="rr")
                nc.vector.reciprocal(rr[:], den[:])
                prb = g.ps.tile([P, 512], f32, tag="ps", name="ps_r")
                nc.tensor.matmul(prb[:DH, :512], g.ones64[:], rr[:],
                                 start=True, stop=True)
                rbc = g.stats.tile([DH, 512], f32, tag="rbc", name="rbc")
                nc.vector.tensor_copy(rbc[:], prb[:DH, :512])
                nc.vector.tensor_mul(out=g.hcat[pair][hs, :],
                                     in0=pav[:DH, :512], in1=rbc[:])


def _phase_cd(g):
    """Wo + residual + LN2 + FFN + output."""
    nc, tc = g.nc, g.tc
    with tc.tile_pool(name="cp", bufs=3) as cp, \
         tc.tile_pool(name="wop", bufs=1) as wop, \
         tc.tile_pool(name="x3p", bufs=1) as x3p:
        wos = wop.tile([P, CC, C], bt16, tag="wos", name="wos")
        nc.sync.dma_start(wos[:], g.wo.rearrange("(c p) f -> p c f", p=P))
        x2T = [x3p.tile([P, 512], f32, tag=f"x2T{m}", name=f"x2T{m}")
               for m in range(CC)]
        for m in range(CC):
            pa = g.ps.tile([P, 512], f32, tag="ps", name="ps_o")
            for c in range(CC):
                nc.tensor.matmul(pa[:, :512], wos[:, c, m * P:(m + 1) * P],
                                 g.hcat[c][:, :], start=(c == 0),
                                 stop=(c == CC - 1))
            nc.vector.scalar_tensor_tensor(
                out=x2T[m][:, :], in0=pa[:, :512],
                scalar=g.sb_vec["bo"][:, m:m + 1], in1=g.x1f[m][:, :],
                op0=OP.add, op1=OP.add)

        x3Tb = [x3p.tile([P, 512], bt16, tag=f"x3Tb{c}", name=f"x3Tb{c}")
                for c in range(CC)]
        x3Tf = [x3p.tile([P, 512], f32, tag=f"x3Tf{c}", name=f"x3Tf{c}")
                for c in range(CC)]
        for rt in range(R // P):
            x2r = cp.tile([P, C], f32, tag="x2r", name="x2r")
            for c in range(CC):
                pt = g.ps.tile([P, 512], f32, tag="ps", name="ps_t2")
                nc.tensor.transpose(pt[:P, :P], x2T[c][:, rt * P:(rt + 1) * P],
                                    g.ident[:])
                nc.scalar.copy(out=x2r[:, c * P:(c + 1) * P], in_=pt[:P, :P])

            def wr2(c, pt, rt=rt):
                nc.scalar.activation(
                    out=x3Tb[c][:, rt * P:(rt + 1) * P], in_=pt,
                    func=AF.Identity, bias=g.be2s[:, c:c + 1],
                    scale=g.g2s[:, c:c + 1])
                nc.scalar.activation(
                    out=x3Tf[c][:, rt * P:(rt + 1) * P], in_=pt,
                    func=AF.Identity, bias=g.be2s[:, c:c + 1],
                    scale=g.g2s[:, c:c + 1])
            _layernorm_tile(g, x2r, wr2)

        # FFN
        with tc.tile_pool(name="dp", bufs=1) as dp:
            h1 = [dp.tile([P, 512], bt16, tag=f"h1_{m}", name=f"h1_{m}")
                  for m in range(MM)]
            for m in range(MM):
                w1m = g.wstr.tile([P, CC, P], bt16, tag="wstr", name="w1m")
                nc.sync.dma_start(
                    w1m[:],
                    g.w1[:, m * P:(m + 1) * P].rearrange("(c p) f -> p c f", p=P))
                p1 = g.ps.tile([P, 512], f32, tag="ps", name="ps_f1")
                for c in range(CC):
                    nc.tensor.matmul(p1[:], w1m[:, c, :], x3Tb[c][:, :],
                                     start=(c == 0), stop=(c == CC - 1))
                nc.scalar.activation(out=h1[m][:, :], in_=p1[:], func=AF.Gelu,
                                     bias=g.sb_vec["b1"][:, m:m + 1], scale=1.0)
            for oc in range(CC):
                w2m = g.wstr.tile([P, MM, P], bt16, tag="wstr", name="w2m")
                nc.sync.dma_start(
                    w2m[:],
                    g.w2[:, oc * P:(oc + 1) * P].rearrange("(k p) f -> p k f", p=P))
                p2 = g.ps.tile([P, 512], f32, tag="ps", name="ps_f2")
                for k in range(MM):
                    nc.tensor.matmul(p2[:], w2m[:, k, :], h1[k][:, :],
                                     start=(k == 0), stop=(k == MM - 1))
                ot = cp.tile([P, R], f32, tag="otile", name="otile")
                nc.vector.scalar_tensor_tensor(
                    out=ot[:], in0=p2[:], scalar=g.sb_vec["b2"][:, oc:oc + 1],
                    in1=x3Tf[oc][:, :], op0=OP.add, op1=OP.add)
                nc.sync.dma_start(g.out[oc], ot[:])


def build_kernel():
    nc = bass.Bass("TRN2", target_bir_lowering=False, num_devices=8)
    g = Ctx()
    g.nc = nc

    g.xc = nc.dram_tensor("xc", [RT, C], f32, kind="ExternalInput").ap()
    g.wq = nc.dram_tensor("wq", [C, C], bt16, kind="ExternalInput").ap()
    g.wk = nc.dram_tensor("wk", [C, C], bt16, kind="ExternalInput").ap()
    g.wv = nc.dram_tensor("wv", [C, C], bt16, kind="ExternalInput").ap()
    g.wo = nc.dram_tensor("wo", [C, C], bt16, kind="ExternalInput").ap()
    g.w1 = nc.dram_tensor("w1", [C, DFF], bt16, kind="ExternalInput").ap()
    g.w2 = nc.dram_tensor("w2", [DFF, C], bt16, kind="ExternalInput").ap()
    g.masks = nc.dram_tensor("masks", [P, 16, QB], bt16,
                             kind="ExternalInput").ap()
    vecs = {}
    for nm, n in [("bq", CC), ("bk", CC), ("bo", CC), ("b1", MM),
                  ("b2", CC), ("g1", CC), ("be1", CC), ("g2", CC), ("be2", CC)]:
        vecs[nm] = nc.dram_tensor(nm, [n, P], f32, kind="ExternalInput").ap()
    g.out = nc.dram_tensor("out", [CC, P, R], f32, kind="ExternalOutput").ap()

    with tile.TileContext(nc) as tc:
        g.tc = tc
        with tc.tile_pool(name="setup", bufs=1) as setup, \
             tc.tile_pool(name="stats", bufs=4) as stats, \
             tc.tile_pool(name="ps", bufs=7, space="PSUM") as ps, \
             tc.tile_pool(name="wstr", bufs=2) as wstr, \
             tc.tile_pool(name="x1f_p", bufs=1) as x1f_p, \
             tc.tile_pool(name="hp", bufs=1) as hp:
            g.stats, g.ps, g.wstr = stats, ps, wstr

            ident = setup.tile([P, P], f32, tag="ident", name="ident")
            make_identity(nc, ident[:])
            g.ident = ident
            g.ones64 = setup.tile([1, DH], f32, tag="ones64", name="ones64")
            nc.vector.memset(g.ones64[:], 1.0)
            g.eps_sb = setup.tile([P, 1], f32, tag="eps", name="eps")
            nc.vector.memset(g.eps_sb[:], EPS)
            g.sb_vec = {}
            for nm, ap_ in vecs.items():
                n = ap_.shape[0]
                t = setup.tile([P, n], f32, tag=f"vec_{nm}", name=f"vec_{nm}")
                nc.sync.dma_start(t[:], ap_.rearrange("c p -> p c"))
                g.sb_vec[nm] = t
            g.g1s, g.be1s = g.sb_vec["g1"], g.sb_vec["be1"]
            g.g2s, g.be2s = g.sb_vec["g2"], g.sb_vec["be2"]

            g.x1f = [x1f_p.tile([P, 512], f32, tag=f"x1f{c}", name=f"x1f{c}")
                     for c in range(CC)]
            g.hcat = [hp.tile([P, 512], bt16, tag=f"hcat{c}", name=f"hcat{c}")
                      for c in range(CC)]

            with tc.tile_pool(name="kvp", bufs=1) as kvp:
                g.kT = [[kvp.tile([P, 512], bt16, tag=f"kT{m}_{rb}",
                                  name=f"kT{m}_{rb}") for rb in range(4)]
                        for m in range(CC)]
                g.vv = [kvp.tile([P, H, DH + 1], bt16, tag=f"vv{kt}",
                                 name=f"vv{kt}") for kt in range(T // P)]
                g.qT = [kvp.tile([P, 512], bt16, tag=f"qT{m}", name=f"qT{m}")
                        for m in range(CC)]
                _phase_a(g)
                _phase_b(g)
            _phase_cd(g)
    _split_sync_waits(nc)
    return nc


_NC_CACHE = None


def _get_nc():
    global _NC_CACHE
    if _NC_CACHE is None:
        _NC_CACHE = build_kernel()
    return _NC_CACHE


def _prep_shared(inputs):
    scale = DH ** -0.5
    Wq = np.asarray(inputs["Wq"], np.float32)
    Wk = np.asarray(inputs["Wk"], np.float32)
    Wv = np.asarray(inputs["Wv"], np.float32)
    Wo = np.asarray(inputs["Wo"], np.float32)
    W1 = np.asarray(inputs["W1"], np.float32)
    W2 = np.asarray(inputs["W2"], np.float32)
    bv_c = np.asarray(inputs["bv"], np.float32).reshape(C)
    shared = {
        "wq": np.ascontiguousarray(
            Wq.transpose(1, 0, 2).reshape(C, C) * scale).astype(bf16),
        "wk": np.ascontiguousarray(
            Wk.transpose(1, 0, 2).reshape(C, C)).astype(bf16),
        "wv": np.ascontiguousarray(
            Wv.transpose(1, 0, 2).reshape(C, C)).astype(bf16),
        "wo": Wo.astype(bf16),
        "w1": W1.astype(bf16),
        "w2": W2.astype(bf16),
        "bq": (np.asarray(inputs["bq"], np.float32).reshape(C) * scale
               ).reshape(CC, P).copy(),
        "bk": np.asarray(inputs["bk"], np.float32).reshape(CC, P).copy(),
        "bo": (np.asarray(inputs["bo"], np.float32) + bv_c @ Wo
               ).reshape(CC, P).copy(),
        "b1": np.asarray(inputs["b1"], np.float32).reshape(MM, P).copy(),
        "b2": np.asarray(inputs["b2"], np.float32).reshape(CC, P).copy(),
        "g1": np.asarray(inputs["gamma1"], np.float32).reshape(CC, P).copy(),
        "be1": np.asarray(inputs["beta1"], np.float32).reshape(CC, P).copy(),
        "g2": np.asarray(inputs["gamma2"], np.float32).reshape(CC, P).copy(),
        "be2": np.asarray(inputs["beta2"], np.float32).reshape(CC, P).copy(),
    }
    return shared


def _core_masks(j):
    """[128, 16, 256] bf16 0/1 masks. kc 0-7 mask the early block's columns
    (block j); kc 8-15 mask the late block's columns (block 7-j)."""
    out = np.zeros((P, 16, QB), np.float32)
    for kc in range(16):
        b = j if kc < 8 else 7 - j
        key = kc * P + np.arange(P)[:, None]          # [128, 1]
        qglob = b * QB + np.arange(QB)[None, :]       # [1, 256]
        out[:, kc, :] = (key <= qglob)
    return out.astype(bf16)


def _make_in_maps(inputs):
    x = np.asarray(inputs["x"], np.float32)
    shared = _prep_shared(inputs)
    in_maps = []
    for c in range(8):
        gg, j = c // 4, c % 4
        xb = x[gg]
        xq = np.concatenate([xb[j * QB:(j + 1) * QB],
                             xb[(7 - j) * QB:(8 - j) * QB]], 0)
        m = dict(shared)
        m["xc"] = np.ascontiguousarray(np.concatenate([xb, xq], 0))
        m["masks"] = _core_masks(j)
        in_maps.append(m)
    return in_maps


def _assemble(results):
    out = np.zeros((B, T, C), np.float32)
    for c in range(8):
        gg, j = c // 4, c % 4
        o = results[c]["out"].reshape(C, R).T  # [512, C] rows = 2 blocks
        out[gg, j * QB:(j + 1) * QB] = o[:QB]
        out[gg, (7 - j) * QB:(8 - j) * QB] = o[QB:]
    return out


def kernel(**inputs):
    in_maps = _make_in_maps(inputs)
    nc = _get_nc()
    res = run_bass_kernel_spmd(nc, in_maps, core_ids=list(range(8)))
    return _assemble(res.results)


# revision 15
# speedup vs baseline: 131.5484x; 131.5484x over previous
"""Trainium2 Bass kernel for a dense transformer block (B=2, T=2048, C=1024,
H=16, DFF=4096), distributed over 8 NeuronCores.

Sharding: 2 batch groups x 4-way query-block sharding. Core c handles batch
g=c//4 and query blocks {j, 7-j} (j=c%4) of 8 blocks of 256 rows. K/V are
computed per-core for the full batch (replicated; no collectives). Causality
is exploited statically: key-chunks 0-7 are needed by both query blocks
(masked only on block-0's columns), chunks 8-15 only by the late block.
The data-dependent causal boundary is applied with per-core 0/1 masks so one
NEFF serves all 8 cores (SPMD).

Activations are kept feature-major ("xT") so every matmul chains without
transposes; layernorm runs row-major with PE transposes between domains.
Softmax denominators ride the AV matmul as an extra ones-column of V.
"""
import numpy as np
import ml_dtypes

import concourse.bass as bass
import concourse.mybir as mybir
import concourse.tile as tile
from concourse.vector_clock import ScopedClock
from concourse.bass_utils import run_bass_kernel_spmd
from concourse.masks import make_identity

bf16 = ml_dtypes.bfloat16
f32 = mybir.dt.float32
bt16 = mybir.dt.bfloat16
AF = mybir.ActivationFunctionType
OP = mybir.AluOpType

B, T, C, H, DH, DFF = 2, 2048, 1024, 16, 64, 4096
P = 128
QB = 256            # rows per query block
R = 512             # own query rows per core
RT = T + R          # ln1 rows per core (full batch + own q rows)
CC = C // P         # 8 feature chunks
MM = DFF // P       # 32 ffn chunks
EPS = 1e-5


# ---------------------------------------------------------------------------
# The walrus build in this container rejects instructions with >1 sync wait.
# Tile's sem assignment can emit several on one instruction; split the excess
# onto same-engine NoOps placed immediately before.
def _patched_drain_and_barrier(self, tick_clock, wait_clock):
    nc = self.nc
    probe = nc.sync.nop(nofuse=True, hint="tail_wait_probe")
    wait_clock.add_sem_waits(probe.ins, ScopedClock({None: tick_clock.global_clock}))
    si = probe.ins.sync_info
    waits = list(si.on_wait) if si is not None else []
    if si is not None:
        si.on_wait = waits[:1]
    for w in waits[1:]:
        n2 = nc.sync.nop(nofuse=True, hint="tail_wait_split")
        n2.ins.sync_info = mybir.SyncInfo(on_wait=[w], on_update=[])
    nc.sync.drain()
    nc.all_engine_barrier()
    assert self.sems is not None
    popped = nc._tile_sem_poison_stack.pop()
    assert popped is self._sem_poison
    nc.clear_and_free_semaphores(list(self.sems.allocated().values()))
    nc.all_engine_barrier()


tile.TileContext._drain_and_barrier = _patched_drain_and_barrier

_MAX_WAITS = 1
_split_counter = [0]


def _split_sync_waits(nc):
    for fn in nc.m.functions:
        for bb in fn.blocks:
            new_insts = []
            for inst in bb.instructions:
                si = getattr(inst, "sync_info", None)
                lim = _MAX_WAITS
                if si is not None and si.on_wait and len(si.on_wait) > lim:
                    waits = list(si.on_wait)
                    keep = waits[-lim:]
                    excess = waits[:-lim]
                    for i in range(0, len(excess), _MAX_WAITS):
                        _split_counter[0] += 1
                        nop = mybir.InstNoOp(
                            name=f"I-wsplit-{_split_counter[0]}", ins=[], outs=[])
                        nop.engine = inst.engine
                        nop.sync_info = mybir.SyncInfo(
                            on_wait=excess[i:i + _MAX_WAITS], on_update=[])
                        new_insts.append(nop)
                    si.on_wait = keep
                new_insts.append(inst)
            bb.instructions = new_insts
# ---------------------------------------------------------------------------


class Ctx:
    pass


def _layernorm_tile(g, xt, out_writes):
    """Row-major LN of xt [128, C] (in place), then transpose chunks and run
    out_writes(c, psum_ap) for each feature chunk c."""
    nc = g.nc
    st = g.stats.tile([P, 2, 6], f32, tag="bnst", name="bnst")
    xv = xt.rearrange("p (s d) -> p s d", s=2)
    for sg in range(2):
        nc.vector.bn_stats(out=st[:, sg, :], in_=xv[:, sg, :])
    mv = g.stats.tile([P, 2], f32, tag="bnmv", name="bnmv")
    nc.vector.bn_aggr(out=mv[:], in_=st[:])
    sq = g.stats.tile([P, 1], f32, tag="bnsq", name="bnsq")
    nc.scalar.activation(out=sq[:], in_=mv[:, 1:2], func=AF.Sqrt,
                         bias=g.eps_sb[:], scale=float(C) / (C - 1))
    rstd = g.stats.tile([P, 1], f32, tag="bnrstd", name="bnrstd")
    nc.vector.reciprocal(rstd[:], sq[:])
    nc.vector.tensor_scalar(out=xt[:], in0=xt[:], scalar1=mv[:, 0:1],
                            scalar2=rstd[:], op0=OP.subtract, op1=OP.mult)
    for c in range(CC):
        pt = g.ps.tile([P, 512], f32, tag="ps", name="ps_t")
        nc.tensor.transpose(pt[:P, :P], xt[:, c * P:(c + 1) * P], g.ident[:])
        out_writes(c, pt[:P, :P])


def _phase_a(g):
    """LN1 + transpose + Q/K/V projections (K/V over the full batch)."""
    nc, tc = g.nc, g.tc
    with tc.tile_pool(name="x1p", bufs=1) as x1p, \
         tc.tile_pool(name="xio", bufs=3) as xio, \
         tc.tile_pool(name="wvp", bufs=1) as wvp:
        # x1T split per 512-row block (rb 0-3 = batch, rb 4 = own q rows)
        x1T = [x1p.tile([P, CC, 512], bt16, tag=f"x1T{rb}", name=f"x1T{rb}")
               for rb in range(5)]
        for kt in range(T // P):
            nc.vector.memset(g.vv[kt][:, :, DH:DH + 1], 1.0)

        for rt in range(RT // P):
            rb, r0 = rt // 4, (rt % 4) * P
            xt = xio.tile([P, C], f32, tag="xin", name="xin")
            nc.sync.dma_start(xt[:], g.xc[rt * P:(rt + 1) * P, :])

            def wr1(c, pt, rb=rb, r0=r0, rt=rt):
                nc.scalar.activation(
                    out=x1T[rb][:, c, r0:r0 + P], in_=pt, func=AF.Identity,
                    bias=g.be1s[:, c:c + 1], scale=g.g1s[:, c:c + 1])
                if rt >= T // P:
                    q0 = (rt - T // P) * P
                    nc.scalar.activation(
                        out=g.x1f[c][:, q0:q0 + P], in_=pt, func=AF.Identity,
                        bias=g.be1s[:, c:c + 1], scale=g.g1s[:, c:c + 1])
            _layernorm_tile(g, xt, wr1)

        # K^T and Q^T projections (feature-major out, weights as lhsT)
        for m in range(CC):
            wkm = g.wstr.tile([P, CC, P], bt16, tag="wstr", name="wkm")
            nc.sync.dma_start(
                wkm[:, :, :],
                g.wk[:, m * P:(m + 1) * P].rearrange("(c p) f -> p c f", p=P))
            for rb in range(4):
                pk = g.ps.tile([P, 512], f32, tag="ps", name="ps_k")
                for c in range(CC):
                    nc.tensor.matmul(pk[:], wkm[:, c, :], x1T[rb][:, c, :],
                                     start=(c == 0), stop=(c == CC - 1))
                nc.vector.tensor_scalar(
                    out=g.kT[m][rb][:, :], in0=pk[:],
                    scalar1=g.sb_vec["bk"][:, m:m + 1], scalar2=None, op0=OP.add)
            wqm = g.wstr.tile([P, CC, P], bt16, tag="wstr", name="wqm")
            nc.sync.dma_start(
                wqm[:, :, :],
                g.wq[:, m * P:(m + 1) * P].rearrange("(c p) f -> p c f", p=P))
            pq = g.ps.tile([P, 512], f32, tag="ps", name="ps_q")
            for c in range(CC):
                nc.tensor.matmul(pq[:], wqm[:, c, :], x1T[4][:, c, :],
                                 start=(c == 0), stop=(c == CC - 1))
            nc.vector.tensor_scalar(
                out=g.qT[m][:, :], in0=pq[:],
                scalar1=g.sb_vec["bq"][:, m:m + 1], scalar2=None, op0=OP.add)

        # V row-major (keys on partitions): lhsT = x1T chunk, rhs = wv
        wvs = wvp.tile([P, CC, C], bt16, tag="wvs", name="wvs")
        nc.sync.dma_start(wvs[:], g.wv.rearrange("(c p) f -> p c f", p=P))
        for kt in range(T // P):
            rb, r0 = kt // 4, (kt % 4) * P
            for half in range(2):
                pv = g.ps.tile([P, 512], f32, tag="ps", name="ps_v")
                for c in range(CC):
                    nc.tensor.matmul(pv[:], x1T[rb][:, c, r0:r0 + P],
                                     wvs[:, c, half * 512:(half + 1) * 512],
                                     start=(c == 0), stop=(c == CC - 1))
                nc.vector.tensor_copy(
                    out=g.vv[kt][:, half * 8:(half + 1) * 8, 0:DH],
                    in_=pv.rearrange("p (h d) -> p h d", h=8))


def _phase_b(g):
    """Attention, both query blocks fused on the free dim (cols 0:256 = early
    block, 256:512 = late block). Key-chunks 0-7 feed both blocks (one N=512
    matmul); chunks 8-15 feed only the late block (N=256)."""
    nc, tc = g.nc, g.tc
    with tc.tile_pool(name="mp", bufs=1) as mp, \
         tc.tile_pool(name="apl", bufs=3) as apl:
        mq = mp.tile([P, 16, QB], bt16, tag="mask", name="mask")
        nc.sync.dma_start(mq[:], g.masks)
        for pair in range(CC):
            for hl in range(2):
                h = 2 * pair + hl
                hs = slice(hl * DH, (hl + 1) * DH)
                aA = apl.tile([P, 8, 512], bt16, tag="aA", name="aA")
                aB = apl.tile([P, 8, QB], bt16, tag="aB", name="aB")
                for kc in range(16):
                    rb, k0 = kc // 4, (kc % 4) * P
                    psc = g.ps.tile([P, 512], f32, tag="ps", name="ps_s")
                    n = 512 if kc < 8 else QB
                    q0 = 0 if kc < 8 else QB
                    nc.tensor.matmul(
                        psc[:, :n], g.kT[pair][rb][hs, k0:k0 + P],
                        g.qT[pair][hs, q0:512],
                        start=True, stop=True, tile_position=(hl * DH, 0))
                    if kc < 8:
                        nc.scalar.activation(out=aA[:, kc, :], in_=psc[:, :512],
                                             func=AF.Exp)
                        nc.vector.tensor_mul(aA[:, kc, 0:QB], aA[:, kc, 0:QB],
                                             mq[:, kc, :])
                    else:
                        nc.scalar.activation(out=aB[:, kc - 8, :],
                                             in_=psc[:, :QB], func=AF.Exp)
                        nc.vector.tensor_mul(aB[:, kc - 8, :], aB[:, kc - 8, :],
                                             mq[:, kc, :])
                pav = g.ps.tile([P, 512], f32, tag="ps", name="ps_av")
                for kc in range(8):
                    nc.tensor.matmul(pav[:DH + 1, :512], g.vv[kc][:, h, :],
                                     aA[:, kc, :], start=(kc == 0), stop=False)
                for kc in range(8, 16):
                    nc.tensor.matmul(pav[:DH + 1, QB:512], g.vv[kc][:, h, :],
                                     aB[:, kc - 8, :], start=False,
                                     stop=(kc == 15))
                den = g.stats.tile([1, 512], f32, tag="den", name="den")
                nc.vector.tensor_copy(den[:], pav[DH:DH + 1, :512])
                rr = g.stats.tile([1, 512], f32, tag="rr", name="rr")
                nc.vector.reciprocal(rr[:], den[:])
                prb = g.ps.tile([P, 512], f32, tag="ps", name="ps_r")
                nc.tensor.matmul(prb[:DH, :512], g.ones64[:], rr[:],
                                 start=True, stop=True)
                rbc = g.stats.tile([DH, 512], f32, tag="rbc", name="rbc")
                nc.vector.tensor_copy(rbc[:], prb[:DH, :512])
                nc.vector.tensor_mul(out=g.hcat[pair][hs, :],
                                     in0=pav[:DH, :512], in1=rbc[:])


def _phase_cd(g):
    """Wo + residual + LN2 + FFN + output."""
    nc, tc = g.nc, g.tc
    with tc.tile_pool(name="cp", bufs=3) as cp, \
         tc.tile_pool(name="wop", bufs=1) as wop, \
         tc.tile_pool(name="x3p", bufs=1) as x3p:
        wos = wop.tile([P, CC, C], bt16, tag="wos", name="wos")
        nc.sync.dma_start(wos[:], g.wo.rearrange("(c p) f -> p c f", p=P))
        x2T = [x3p.tile([P, 512], f32, tag=f"x2T{m}", name=f"x2T{m}")
               for m in range(CC)]
        for m in range(CC):
            pa = g.ps.tile([P, 512], f32, tag="ps", name="ps_o")
            for c in range(CC):
                nc.tensor.matmul(pa[:, :512], wos[:, c, m * P:(m + 1) * P],
                                 g.hcat[c][:, :], start=(c == 0),
                                 stop=(c == CC - 1))
            nc.vector.scalar_tensor_tensor(
                out=x2T[m][:, :], in0=pa[:, :512],
                scalar=g.sb_vec["bo"][:, m:m + 1], in1=g.x1f[m][:, :],
                op0=OP.add, op1=OP.add)

        x3Tb = [x3p.tile([P, 512], bt16, tag=f"x3Tb{c}", name=f"x3Tb{c}")
                for c in range(CC)]
        x3Tf = [x3p.tile([P, 512], f32, tag=f"x3Tf{c}", name=f"x3Tf{c}")
                for c in range(CC)]
        for rt in range(R // P):
            x2r = cp.tile([P, C], f32, tag="x2r", name="x2r")
            for c in range(CC):
                pt = g.ps.tile([P, 512], f32, tag="ps", name="ps_t2")
                nc.tensor.transpose(pt[:P, :P], x2T[c][:, rt * P:(rt + 1) * P],
                                    g.ident[:])
                nc.scalar.copy(out=x2r[:, c * P:(c + 1) * P], in_=pt[:P, :P])

            def wr2(c, pt, rt=rt):
                nc.scalar.activation(
                    out=x3Tb[c][:, rt * P:(rt + 1) * P], in_=pt,
                    func=AF.Identity, bias=g.be2s[:, c:c + 1],
                    scale=g.g2s[:, c:c + 1])
                nc.scalar.activation(
                    out=x3Tf[c][:, rt * P:(rt + 1) * P], in_=pt,
                    func=AF.Identity, bias=g.be2s[:, c:c + 1],
                    scale=g.g2s[:, c:c + 1])
            _layernorm_tile(g, x2r, wr2)

        # FFN
        with tc.tile_pool(name="dp", bufs=1) as dp:
            h1 = [dp.tile([P, 512], bt16, tag=f"h1_{m}", name=f"h1_{m}")
                  for m in range(MM)]
            for m in range(MM):
                w1m = g.wstr.tile([P, CC, P], bt16, tag="wstr", name="w1m")
                nc.sync.dma_start(
                    w1m[:],
                    g.w1[:, m * P:(m + 1) * P].rearrange("(c p) f -> p c f", p=P))
                p1 = g.ps.tile([P, 512], f32, tag="ps", name="ps_f1")
                for c in range(CC):
                    nc.tensor.matmul(p1[:], w1m[:, c, :], x3Tb[c][:, :],
                                     start=(c == 0), stop=(c == CC - 1))
                nc.scalar.activation(out=h1[m][:, :], in_=p1[:], func=AF.Gelu,
                                     bias=g.sb_vec["b1"][:, m:m + 1], scale=1.0)
            for oc in range(CC):
                w2m = g.wstr.tile([P, MM, P], bt16, tag="wstr", name="w2m")
                nc.sync.dma_start(
                    w2m[:],
                    g.w2[:, oc * P:(oc + 1) * P].rearrange("(k p) f -> p k f", p=P))
                p2 = g.ps.tile([P, 512], f32, tag="ps", name="ps_f2")
                for k in range(MM):
                    nc.tensor.matmul(p2[:], w2m[:, k, :], h1[k][:, :],
                                     start=(k == 0), stop=(k == MM - 1))
                ot = cp.tile([P, R], f32, tag="otile", name="otile")
                nc.vector.scalar_tensor_tensor(
                    out=ot[:], in0=p2[:], scalar=g.sb_vec["b2"][:, oc:oc + 1],
                    in1=x3Tf[oc][:, :], op0=OP.add, op1=OP.add)
                nc.sync.dma_start(g.out[oc], ot[:])


def build_kernel():
    nc = bass.Bass("TRN2", target_bir_lowering=False, num_devices=8)
    g = Ctx()
    g.nc = nc

    g.xc = nc.dram_tensor("xc", [RT, C], f32, kind="ExternalInput").ap()
    g.wq = nc.dram_tensor("wq", [C, C], bt16, kind="ExternalInput").ap()
    g.wk = nc.dram_tensor("wk", [C, C], bt16, kind="ExternalInput").ap()
    g.wv = nc.dram_tensor("wv", [C, C], bt16, kind="ExternalInput").ap()
    g.wo = nc.dram_tensor("wo", [C, C], bt16, kind="ExternalInput").ap()
    g.w1 = nc.dram_tensor("w1", [C, DFF], bt16, kind="ExternalInput").ap()
    g.w2 = nc.dram_tensor("w2", [DFF, C], bt16, kind="ExternalInput").ap()
    g.masks = nc.dram_tensor("masks", [P, 16, QB], bt16,
                             kind="ExternalInput").ap()
    vecs = {}
    for nm, n in [("bq", CC), ("bk", CC), ("bo", CC), ("b1", MM),
                  ("b2", CC), ("g1", CC), ("be1", CC), ("g2", CC), ("be2", CC)]:
        vecs[nm] = nc.dram_tensor(nm, [n, P], f32, kind="ExternalInput").ap()
    g.out = nc.dram_tensor("out", [CC, P, R], f32, kind="ExternalOutput").ap()

    with tile.TileContext(nc) as tc:
        g.tc = tc
        with tc.tile_pool(name="setup", bufs=1) as setup, \
             tc.tile_pool(name="stats", bufs=4) as stats, \
             tc.tile_pool(name="ps", bufs=8, space="PSUM") as ps, \
             tc.tile_pool(name="wstr", bufs=2) as wstr, \
             tc.tile_pool(name="x1f_p", bufs=1) as x1f_p, \
             tc.tile_pool(name="hp", bufs=1) as hp:
            g.stats, g.ps, g.wstr = stats, ps, wstr

            ident = setup.tile([P, P], f32, tag="ident", name="ident")
            make_identity(nc, ident[:])
            g.ident = ident
            g.ones64 = setup.tile([1, DH], f32, tag="ones64", name="ones64")
            nc.vector.memset(g.ones64[:], 1.0)
            g.eps_sb = setup.tile([P, 1], f32, tag="eps", name="eps")
            nc.vector.memset(g.eps_sb[:], EPS)
            g.sb_vec = {}
            for nm, ap_ in vecs.items():
                n = ap_.shape[0]
                t = setup.tile([P, n], f32, tag=f"vec_{nm}", name=f"vec_{nm}")
                nc.sync.dma_start(t[:], ap_.rearrange("c p -> p c"))
                g.sb_vec[nm] = t
            g.g1s, g.be1s = g.sb_vec["g1"], g.sb_vec["be1"]
            g.g2s, g.be2s = g.sb_vec["g2"], g.sb_vec["be2"]

            g.x1f = [x1f_p.tile([P, 512], f32, tag=f"x1f{c}", name=f"x1f{c}")
                     for c in range(CC)]
            g.hcat = [hp.tile([P, 512], bt16, tag=f"hcat{c}", name=f"hcat{c}")
                      for c in range(CC)]

            with tc.tile_pool(name="kvp", bufs=1) as kvp:
                g.kT = [[kvp.tile([P, 512], bt16, tag=f"kT{m}_{rb}",
                                  name=f"kT{m}_{rb}") for rb in range(4)]
                        for m in range(CC)]
                g.vv = [kvp.tile([P, H, DH + 1], bt16, tag=f"vv{kt}",
                                 name=f"vv{kt}") for kt in range(T // P)]
                g.qT = [kvp.tile([P, 512], bt16, tag=f"qT{m}", name=f"qT{m}")
                        for m in range(CC)]
                _phase_a(g)
                _phase_b(g)
            _phase_cd(g)
    _split_sync_waits(nc)
    return nc


_NC_CACHE = None


def _get_nc():
    global _NC_CACHE
    if _NC_CACHE is None:
        _NC_CACHE = build_kernel()
    return _NC_CACHE


def _prep_shared(inputs):
    scale = DH ** -0.5
    Wq = np.asarray(inputs["Wq"], np.float32)
    Wk = np.asarray(inputs["Wk"], np.float32)
    Wv = np.asarray(inputs["Wv"], np.float32)
    Wo = np.asarray(inputs["Wo"], np.float32)
    W1 = np.asarray(inputs["W1"], np.float32)
    W2 = np.asarray(inputs["W2"], np.float32)
    bv_c = np.asarray(inputs["bv"], np.float32).reshape(C)
    shared = {
        "wq": np.ascontiguousarray(
            Wq.transpose(1, 0, 2).reshape(C, C) * scale).astype(bf16),
        "wk": np.ascontiguousarray(
            Wk.transpose(1, 0, 2).reshape(C, C)).astype(bf16),
        "wv": np.ascontiguousarray(
            Wv.transpose(1, 0, 2).reshape(C, C)).astype(bf16),
        "wo": Wo.astype(bf16),
        "w1": W1.astype(bf16),
        "w2": W2.astype(bf16),
        "bq": (np.asarray(inputs["bq"], np.float32).reshape(C) * scale
               ).reshape(CC, P).copy(),
        "bk": np.asarray(inputs["bk"], np.float32).reshape(CC, P).copy(),
        "bo": (np.asarray(inputs["bo"], np.float32) + bv_c @ Wo
               ).reshape(CC, P).copy(),
        "b1": np.asarray(inputs["b1"], np.float32).reshape(MM, P).copy(),
        "b2": np.asarray(inputs["b2"], np.float32).reshape(CC, P).copy(),
        "g1": np.asarray(inputs["gamma1"], np.float32).reshape(CC, P).copy(),
        "be1": np.asarray(inputs["beta1"], np.float32).reshape(CC, P).copy(),
        "g2": np.asarray(inputs["gamma2"], np.float32).reshape(CC, P).copy(),
        "be2": np.asarray(inputs["beta2"], np.float32).reshape(CC, P).copy(),
    }
    return shared


def _core_masks(j):
    """[128, 16, 256] bf16 0/1 masks. kc 0-7 mask the early block's columns
    (block j); kc 8-15 mask the late block's columns (block 7-j)."""
    out = np.zeros((P, 16, QB), np.float32)
    for kc in range(16):
        b = j if kc < 8 else 7 - j
        key = kc * P + np.arange(P)[:, None]          # [128, 1]
        qglob = b * QB + np.arange(QB)[None, :]       # [1, 256]
        out[:, kc, :] = (key <= qglob)
    return out.astype(bf16)


def _make_in_maps(inputs):
    x = np.asarray(inputs["x"], np.float32)
    shared = _prep_shared(inputs)
    in_maps = []
    for c in range(8):
        gg, j = c // 4, c % 4
        xb = x[gg]
        xq = np.concatenate([xb[j * QB:(j + 1) * QB],
                             xb[(7 - j) * QB:(8 - j) * QB]], 0)
        m = dict(shared)
        m["xc"] = np.ascontiguousarray(np.concatenate([xb, xq], 0))
        m["masks"] = _core_masks(j)
        in_maps.append(m)
    return in_maps


def _assemble(results):
    out = np.zeros((B, T, C), np.float32)
    for c in range(8):
        gg, j = c // 4, c % 4
        o = results[c]["out"].reshape(C, R).T  # [512, C] rows = 2 blocks
        out[gg, j * QB:(j + 1) * QB] = o[:QB]
        out[gg, (7 - j) * QB:(8 - j) * QB] = o[QB:]
    return out


def kernel(**inputs):
    in_maps = _make_in_maps(inputs)
    nc = _get_nc()
    res = run_bass_kernel_spmd(nc, in_maps, core_ids=list(range(8)))
    return _assemble(res.results)
